# revision 1
# baseline (speedup 1.0000x reference)
"""MoE logistic regression kernel for 8 Trainium2 NeuronCores.

Math (after dead-code elimination of the reference's unused router path):
    noise_logits = x @ noise_w.T + noise_b            # [B, E]
    top8 = top_k(noise_logits, 8)
    gates = softmax over the top-8 entries (others 0)
    expert = sigmoid(x @ expert_w.T + expert_b)       # [B, E]
    out[b] = sum_e gates[b,e] * expert[b,e]           # [B, 1]

Sharding: batch split 8 ways (2048 rows/core); weights replicated.

Key implementation choices:
- x is transposed on the host so each core streams contiguous [D, BC]
  chunks with D on partitions; no on-chip transpose of x.
- x and w are split into fp16 (hi, lo) pairs on the host (exact to ~22
  mantissa bits). The matmul runs 3 fp16 passes (hi@wh + lo@wh + hi@wl)
  accumulating in fp32 PSUM: ~fp32 accuracy at 3/4 the fp32 PE cost.
  (The top-8 selection margins require ~1e-6 logit accuracy: the
  smallest 8th/9th gap over the whole fixed batch is 8.8e-6.)
- noise_w/expert_w are concatenated into one 128-wide stationary operand
  so x streams through the PE once per (chunk, pass) for both matmuls;
  biases are added per-partition by the ACT epilogue ops.
- top-8 per row via the DVE Max8 + MatchReplace8 instructions; gates via
  exp(v - m1) with the (e_all - e_zap) trick which is exactly zero off
  the top-8; final dot + 1/Z normalization per 128-row tile.
"""

import sys

import numpy as np

if "/opt/trn_rl_repo" not in sys.path:
    sys.path.insert(0, "/opt/trn_rl_repo")

B, D, E, TOPK, NCORES = 16384, 4096, 64, 8, 8
BC = B // NCORES      # batch rows per core
BT = 512              # batch tile (one PSUM bank of fp32)
NT = BC // BT         # batch tiles per core
NK = D // 128         # contraction chunks
NEG_BIG = -1e30

_cached = {}


def _build_program(mm_dtype="fp16x2"):
    import concourse.bass as bass
    import concourse.tile as tile
    from concourse import bacc, mybir
    from concourse.masks import make_identity

    f32 = mybir.dt.float32
    f16 = mybir.dt.float16
    split = mm_dtype == "fp16x2"
    wdt = f16 if split else getattr(mybir.dt, mm_dtype)
    act = mybir.ActivationFunctionType

    nc = bacc.Bacc("TRN2", target_bir_lowering=False, debug=False)
    if split:
        # x as fp16 (hi, lo): [D, NT, 2, BT]; w pair pre-swizzled so the
        # SBUF image [128, NK*2*128] is one contiguous DMA.
        xt = nc.dram_tensor("xt", [D, NT, 2, BT], f16, kind="ExternalInput").ap()
        wt = nc.dram_tensor("wt", [128, NK * 2 * 128], f16,
                            kind="ExternalInput").ap()
    else:
        xt = nc.dram_tensor("xt", [D, NT, BT], f32, kind="ExternalInput").ap()
        wt = nc.dram_tensor("wt", [128, NK * 128], f32, kind="ExternalInput").ap()
    bb = nc.dram_tensor("bb", [128, 1], f32, kind="ExternalInput").ap()
    out = nc.dram_tensor("out", [BC, 1], f32, kind="ExternalOutput").ap()

    with tile.TileContext(nc) as tc:
        with (
            tc.tile_pool(name="consts", bufs=1) as consts,
            tc.tile_pool(name="xpool", bufs=6) as xpool,
            tc.tile_pool(name="eppool", bufs=4) as eppool,
            tc.tile_pool(name="small", bufs=3) as small,
            tc.tile_pool(name="psacc", bufs=1, space=bass.MemorySpace.PSUM) as psacc,
            tc.tile_pool(name="pstr", bufs=2, space=bass.MemorySpace.PSUM) as pstr,
            tc.tile_pool(name="psfin", bufs=1, space=bass.MemorySpace.PSUM) as psfin,
        ):
            # ---- constants ----
            if split:
                wt_first = consts.tile([128, 2, 2, 128], wdt)
                nc.scalar.dma_start(out=wt_first, in_=wt[:, 0:2 * 2 * 128]
                                    .rearrange("p (nk two m) -> p nk two m",
                                               nk=2, two=2))
                wt_sb = consts.tile([128, NK - 2, 2, 128], wdt)
                nc.scalar.dma_start(out=wt_sb, in_=wt[:, 2 * 2 * 128:]
                                    .rearrange("p (nk two m) -> p nk two m",
                                               nk=NK - 2, two=2))
            else:
                wt_sb = consts.tile([128, NK, 128], wdt)
                nc.scalar.dma_start(out=wt_sb, in_=wt)
            bb_sb = consts.tile([128, 1], f32)
            nc.scalar.dma_start(out=bb_sb, in_=bb)
            ident = consts.tile([128, 128], f32)
            make_identity(nc, ident)
            # warm the ACT function tables during the DMA/matmul phase so the
            # first epilogue ops don't pay serial LoadActFuncSet latency
            warm = consts.tile([1, 1], f32)
            nc.vector.memset(warm, 0.0)
            nc.scalar.add(warm, warm, bb_sb[0:1, :])
            nc.scalar.activation(warm, warm, func=act.Sigmoid,
                                 bias=bb_sb[0:1, :])
            nc.scalar.activation(warm, warm, func=act.Exp)
            nc.scalar.mul(warm, warm, 1.0)
            final_sb = consts.tile([128, NT * 4], f32)

            # ---- matmuls: acc[t][0:64,:] = noise logits.T (pre-bias),
            #               acc[t][64:128,:] = expert logits.T (pre-bias)
            accs = [psacc.tile([128, BT], f32, tag=f"acc{t}", name=f"acc{t}")
                    for t in range(NT)]
            if split:
                # pair k-chunks: one 2MB DMA covers chunks 2kk and 2kk+1
                xview = xt.rearrange("(nkk two p) nt t b -> nkk p two nt t b",
                                     p=128, two=2)
                for kk in range(NK // 2):
                    xk = xpool.tile([128, 2, NT, 2, BT], wdt, tag="xk")
                    nc.sync.dma_start(out=xk, in_=xview[kk])
                    for c in range(2):
                        k = 2 * kk + c
                        wsrc = wt_first if k < 2 else wt_sb
                        ki = k if k < 2 else k - 2
                        wh = wsrc[:, ki, 0, :]
                        wl = wsrc[:, ki, 1, :]
                        for t in range(NT):
                            nc.tensor.matmul(accs[t], lhsT=wh,
                                             rhs=xk[:, c, t, 0, :],
                                             start=(k == 0), stop=False)
                            nc.tensor.matmul(accs[t], lhsT=wh,
                                             rhs=xk[:, c, t, 1, :],
                                             start=False, stop=False)
                            nc.tensor.matmul(accs[t], lhsT=wl,
                                             rhs=xk[:, c, t, 0, :],
                                             start=False,
                                             stop=(k == NK - 1))
            else:
                xview = xt.rearrange("(nk p) nt b -> nk p nt b", p=128)
                for k in range(NK):
                    xk = xpool.tile([128, NT, BT], wdt, tag="xk")
                    nc.sync.dma_start(out=xk, in_=xview[k])
                    for t in range(NT):
                        nc.tensor.matmul(accs[t], lhsT=wt_sb[:, k, :],
                                         rhs=xk[:, t, :],
                                         start=(k == 0), stop=(k == NK - 1))

            # ---- epilogue: pass 1 emits all bias/sigmoid + transposes so
            # the ACT FIFO isn't blocked by tile t's exp stream when tile
            # t+1's head ops become ready; pass 2 does the per-tile math.
            ps_nes = []
            for t in range(NT):
                noiseT = eppool.tile([64, BT], f32, tag="noiseT")
                nc.scalar.add(noiseT, accs[t][0:64, :], bb_sb[0:64, :])
                eoT = eppool.tile([64, BT], f32, tag="eoT")
                nc.scalar.activation(eoT, accs[t][64:128, :],
                                     func=act.Sigmoid, bias=bb_sb[64:128, :])
                # transpose to batch-major: [128 batch, j | 4+j, 64]
                ps_ne = pstr.tile([128, 8, 64], f32, tag="ps_ne",
                                  name=f"ps_ne{t}")
                for j in range(4):
                    nc.tensor.transpose(ps_ne[:, j, :],
                                        noiseT[:, j * 128:(j + 1) * 128],
                                        ident[0:64, 0:64])
                    nc.tensor.transpose(ps_ne[:, 4 + j, :],
                                        eoT[:, j * 128:(j + 1) * 128],
                                        ident[0:64, 0:64])
                ps_nes.append(ps_ne)
            for t in range(NT):
                ps_ne = ps_nes[t]
                e_all = small.tile([128, 4, 64], f32, tag="e_all")
                e_zap = small.tile([128, 4, 64], f32, tag="e_zap")
                zsum = small.tile([128, 4], f32, tag="zsum")
                for j in range(4):
                    v = ps_ne[:, j, :]
                    tv = small.tile([128, 8], f32, tag="tv")
                    nc.vector.max(tv, v)                      # top-8, descending
                    zap = small.tile([128, 64], f32, tag="zap")
                    nc.vector.match_replace(out=zap, in_to_replace=tv,
                                            in_values=v, imm_value=NEG_BIG)
                    negm1 = small.tile([128, 1], f32, tag="negm1")
                    nc.scalar.mul(negm1, tv[:, 0:1], -1.0)
                    nc.scalar.activation(e_all[:, j, :], v, func=act.Exp,
                                         bias=negm1)
                    nc.scalar.activation(e_zap[:, j, :], zap, func=act.Exp,
                                         bias=negm1)
                # g = exp(v-m1) on top-8 positions, exactly 0 elsewhere;
                # grouped DVE math over all four 128-row subtiles at once
                g = small.tile([128, 4, 64], f32, tag="g")
                nc.vector.tensor_sub(g, e_all, e_zap)
                nc.vector.reduce_sum(zsum, g, axis=mybir.AxisListType.X)
                scr = small.tile([128, 4, 64], f32, tag="scr")
                nc.vector.tensor_mul(scr, g, ps_ne[:, 4:8, :])
                s4 = small.tile([128, 4], f32, tag="s4")
                nc.vector.reduce_sum(s4, scr, axis=mybir.AxisListType.X)
                rz = small.tile([128, 4], f32, tag="rz")
                nc.vector.reciprocal(rz, zsum)
                nc.vector.tensor_mul(final_sb[:, t * 4:(t + 1) * 4], s4, rz)

            # ---- output: [128, 16] -> [16, 128] -> DRAM [2048, 1] ----
            fin_ps = psfin.tile([16, 128], f32, tag="fin")
            nc.tensor.transpose(fin_ps, final_sb, ident)
            fin_t = eppool.tile([16, 128], f32, tag="fint")
            nc.scalar.copy(fin_t, fin_ps)
            nc.sync.dma_start(out=out.rearrange("(c p) o -> c (p o)", p=128),
                              in_=fin_t)

    nc.compile()
    return nc


def get_program(mm_dtype="fp16x2"):
    if mm_dtype not in _cached:
        _cached[mm_dtype] = _build_program(mm_dtype)
    return _cached[mm_dtype]


def make_in_maps(x, noise_w, noise_b, expert_w, expert_b, mm_dtype="fp16x2"):
    """Host-side sharding: per-core transposed x slice + replicated weights."""
    w_comb = np.concatenate([noise_w, expert_w], axis=0).astype(np.float32)  # [128, D]
    wt32 = np.ascontiguousarray(w_comb.T)                                    # [D, 128]
    bb = np.concatenate([noise_b, expert_b]).astype(np.float32).reshape(128, 1)
    if mm_dtype == "fp16x2":
        wh = wt32.astype(np.float16)
        wl = (wt32 - wh.astype(np.float32)).astype(np.float16)
        wp = np.stack([wh, wl], axis=1)                   # [D, 2, 128]
        # SBUF image: partition p holds [nk, 2, 128] for rows nk*128+p
        wt = np.ascontiguousarray(
            wp.reshape(NK, 128, 2, 128).transpose(1, 0, 2, 3).reshape(128, -1))
    else:
        wt = np.ascontiguousarray(
            wt32.reshape(NK, 128, 128).transpose(1, 0, 2).reshape(128, -1))
    in_maps = []
    for c in range(NCORES):
        xs = np.ascontiguousarray(x[c * BC:(c + 1) * BC, :].T)               # [D, BC]
        if mm_dtype == "fp16x2":
            xh = xs.astype(np.float16)
            xl = (xs - xh.astype(np.float32)).astype(np.float16)
            xs = np.ascontiguousarray(
                np.stack([xh.reshape(D, NT, BT), xl.reshape(D, NT, BT)],
                         axis=2))                                            # [D,NT,2,BT]
        else:
            xs = np.ascontiguousarray(xs.reshape(D, NT, BT))
        in_maps.append({"xt": xs, "wt": wt, "bb": bb})
    return in_maps


def kernel(x, noise, router_w, router_b, noise_w, noise_b, expert_w, expert_b,
           _trace=False):
    from concourse.bass_utils import run_bass_kernel_spmd

    x = np.asarray(x, dtype=np.float32)
    nc = get_program()
    in_maps = make_in_maps(x, np.asarray(noise_w), np.asarray(noise_b),
                           np.asarray(expert_w), np.asarray(expert_b))
    res = run_bass_kernel_spmd(nc, in_maps, core_ids=list(range(NCORES)),
                               trace=_trace)
    out = np.concatenate([r["out"] for r in res.results], axis=0)
    if _trace:
        kernel.last_results = res
    return out



# revision 2
# speedup vs baseline: 1.9455x; 1.9455x over previous
"""MoE logistic regression kernel for 8 Trainium2 NeuronCores.

Math (after dead-code elimination of the reference's unused router path):
    noise_logits = x @ noise_w.T + noise_b            # [B, E]
    top8 = top_k(noise_logits, 8)
    gates = softmax over the top-8 entries (others 0)
    expert = sigmoid(x @ expert_w.T + expert_b)       # [B, E]
    out[b] = sum_e gates[b,e] * expert[b,e]           # [B, 1]

Sharding: batch split 8 ways (2048 rows/core); weights replicated.

Implementation: single-pass fp16 matmul (x and w rounded to fp16 on the
host). Logit error is ~2.3e-4 which flips the 8th/9th expert on ~24 of
16384 rows; measured end-to-end l2 rel err ~1.2e-3, well inside the 2e-2
gate, at half the DMA bytes and a third of the PE work of an fp16
hi/lo-split scheme.

The stream is batch-tile-major: each 512-row tile's full contraction
(4 MB) arrives as 4x 1MB DMAs, its matmuls accumulate in one PSUM bank,
and its top-8/softmax/sigmoid epilogue runs on ACT/DVE while the next
tile streams. Only the last tile's epilogue is a serial tail.
"""

import sys

import numpy as np

if "/opt/trn_rl_repo" not in sys.path:
    sys.path.insert(0, "/opt/trn_rl_repo")

B, D, E, TOPK, NCORES = 16384, 4096, 64, 8, 8
BC = B // NCORES      # batch rows per core
BT = 512              # batch tile (one PSUM bank of fp32)
NT = BC // BT         # batch tiles per core
NK = D // 128         # contraction chunks
G = 8                 # k-chunks per x DMA (1 MB transfers)
NG = NK // G
NEG_BIG = -1e30

_cached = {}


def _build_program():
    import concourse.bass as bass
    import concourse.tile as tile
    from concourse import bacc, mybir
    from concourse.masks import make_identity

    f32 = mybir.dt.float32
    f16 = mybir.dt.float16
    act = mybir.ActivationFunctionType

    nc = bacc.Bacc("TRN2", target_bir_lowering=False, debug=False)
    # x, fp16, tile-major: [t, gg, p, g*BT] with contraction row gg*G*128
    # + g*128 + p, batch col t*BT + b -> every DMA is 128 partitions x 8KB
    # contiguous.
    xt = nc.dram_tensor("xt", [NT, NG, 128, G * BT], f16,
                        kind="ExternalInput").ap()
    # weights: partition p holds, for each k, the 128 output cols of
    # contraction row k*128+p. Split so group 0 can load first.
    wt0 = nc.dram_tensor("wt0", [128, G * 128], f16, kind="ExternalInput").ap()
    wt1 = nc.dram_tensor("wt1", [128, (NK - G) * 128], f16,
                         kind="ExternalInput").ap()
    bb = nc.dram_tensor("bb", [128, 1], f32, kind="ExternalInput").ap()
    out = nc.dram_tensor("out", [BC, 1], f32, kind="ExternalOutput").ap()

    with tile.TileContext(nc) as tc:
        with (
            tc.tile_pool(name="consts", bufs=1) as consts,
            tc.tile_pool(name="xpool", bufs=4) as xpool,
            tc.tile_pool(name="eppool", bufs=4) as eppool,
            tc.tile_pool(name="small", bufs=3) as small,
            tc.tile_pool(name="psacc", bufs=1, space=bass.MemorySpace.PSUM) as psacc,
            tc.tile_pool(name="pstr", bufs=2, space=bass.MemorySpace.PSUM) as pstr,
            tc.tile_pool(name="psfin", bufs=1, space=bass.MemorySpace.PSUM) as psfin,
        ):
            # ---- constants ----
            w0_sb = consts.tile([128, G, 128], f16)
            nc.sync.dma_start(out=w0_sb,
                              in_=wt0.rearrange("p (g m) -> p g m", g=G))
            w1_sb = consts.tile([128, NK - G, 128], f16)
            nc.sync.dma_start(out=w1_sb,
                              in_=wt1.rearrange("p (g m) -> p g m", g=NK - G))
            bb_sb = consts.tile([128, 1], f32)
            nc.scalar.dma_start(out=bb_sb, in_=bb)
            ident = consts.tile([128, 128], f32)
            make_identity(nc, ident)
            # warm the ACT function tables during the DMA phase
            warm = consts.tile([1, 1], f32)
            nc.vector.memset(warm, 0.0)
            nc.scalar.add(warm, warm, bb_sb[0:1, :])
            nc.scalar.activation(warm, warm, func=act.Sigmoid,
                                 bias=bb_sb[0:1, :])
            nc.scalar.activation(warm, warm, func=act.Exp)
            nc.scalar.mul(warm, warm, 1.0)
            final_sb = consts.tile([128, NT * 4], f32)

            accs = [psacc.tile([128, BT], f32, tag=f"acc{t}", name=f"acc{t}")
                    for t in range(NT)]

            for t in range(NT):
                # ---- stream tile t's contraction, accumulate logits.T ----
                # acc[0:64,:]  = noise logits.T (pre-bias)
                # acc[64:128,:] = expert logits.T (pre-bias)
                for gg in range(NG):
                    xk = xpool.tile([128, G, BT], f16, tag="xk")
                    nc.sync.dma_start(
                        out=xk, in_=xt[t, gg].rearrange("p (g b) -> p g b", g=G))
                    for g in range(G):
                        k = gg * G + g
                        w = w0_sb[:, g, :] if gg == 0 else w1_sb[:, k - G, :]
                        nc.tensor.matmul(accs[t], lhsT=w, rhs=xk[:, g, :],
                                         start=(k == 0), stop=(k == NK - 1))

                # ---- epilogue for tile t (overlaps tile t+1's stream) ----
                noiseT = eppool.tile([64, BT], f32, tag="noiseT")
                nc.scalar.add(noiseT, accs[t][0:64, :], bb_sb[0:64, :])
                eoT = eppool.tile([64, BT], f32, tag="eoT")
                nc.scalar.activation(eoT, accs[t][64:128, :],
                                     func=act.Sigmoid, bias=bb_sb[64:128, :])
                # transpose to batch-major: [128 batch, j | 4+j, 64]
                ps_ne = pstr.tile([128, 8, 64], f32, tag="ps_ne",
                                  name=f"ps_ne{t}")
                for j in range(4):
                    nc.tensor.transpose(ps_ne[:, j, :],
                                        noiseT[:, j * 128:(j + 1) * 128],
                                        ident[0:64, 0:64])
                    nc.tensor.transpose(ps_ne[:, 4 + j, :],
                                        eoT[:, j * 128:(j + 1) * 128],
                                        ident[0:64, 0:64])
                e_all = small.tile([128, 4, 64], f32, tag="e_all")
                e_zap = small.tile([128, 4, 64], f32, tag="e_zap")
                zsum = small.tile([128, 4], f32, tag="zsum")
                for j in range(4):
                    v = ps_ne[:, j, :]
                    tv = small.tile([128, 8], f32, tag="tv")
                    nc.vector.max(tv, v)                      # top-8, descending
                    zap = small.tile([128, 64], f32, tag="zap")
                    nc.vector.match_replace(out=zap, in_to_replace=tv,
                                            in_values=v, imm_value=NEG_BIG)
                    negm1 = small.tile([128, 1], f32, tag="negm1")
                    nc.scalar.mul(negm1, tv[:, 0:1], -1.0)
                    nc.scalar.activation(e_all[:, j, :], v, func=act.Exp,
                                         bias=negm1)
                    nc.scalar.activation(e_zap[:, j, :], zap, func=act.Exp,
                                         bias=negm1)
                # g = exp(v-m1) on top-8 positions, exactly 0 elsewhere
                gg_t = small.tile([128, 4, 64], f32, tag="gg_t")
                nc.vector.tensor_sub(gg_t, e_all, e_zap)
                nc.vector.reduce_sum(zsum, gg_t, axis=mybir.AxisListType.X)
                scr = small.tile([128, 4, 64], f32, tag="scr")
                nc.vector.tensor_mul(scr, gg_t, ps_ne[:, 4:8, :])
                s4 = small.tile([128, 4], f32, tag="s4")
                nc.vector.reduce_sum(s4, scr, axis=mybir.AxisListType.X)
                rz = small.tile([128, 4], f32, tag="rz")
                nc.vector.reciprocal(rz, zsum)
                nc.vector.tensor_mul(final_sb[:, t * 4:(t + 1) * 4], s4, rz)

            # ---- output: [128, 16] -> [16, 128] -> DRAM [2048, 1] ----
            fin_ps = psfin.tile([16, 128], f32, tag="fin")
            nc.tensor.transpose(fin_ps, final_sb, ident)
            fin_t = eppool.tile([16, 128], f32, tag="fint")
            nc.scalar.copy(fin_t, fin_ps)
            nc.sync.dma_start(out=out.rearrange("(c p) o -> c (p o)", p=128),
                              in_=fin_t)

    nc.compile()
    return nc


def get_program():
    if "prog" not in _cached:
        _cached["prog"] = _build_program()
    return _cached["prog"]


def make_in_maps(x, noise_w, noise_b, expert_w, expert_b):
    """Host-side sharding: per-core transposed fp16 x slice + weights."""
    w_comb = np.concatenate([noise_w, expert_w], axis=0).astype(np.float32)  # [128, D]
    wt32 = np.ascontiguousarray(w_comb.T).astype(np.float16)                 # [D, 128]
    # partition p holds [nk, 128] for contraction rows nk*128+p
    wt = np.ascontiguousarray(
        wt32.reshape(NK, 128, 128).transpose(1, 0, 2).reshape(128, -1))
    wt0 = np.ascontiguousarray(wt[:, :G * 128])
    wt1 = np.ascontiguousarray(wt[:, G * 128:])
    bb = np.concatenate([noise_b, expert_b]).astype(np.float32).reshape(128, 1)
    in_maps = []
    for c in range(NCORES):
        xs = np.ascontiguousarray(x[c * BC:(c + 1) * BC, :].T).astype(np.float16)
        # [D, BC] -> [NT, NG, 128, G*BT]: [k*128+p, t*BT+b] -> [t, gg, p, g*BT+b]
        xr = np.ascontiguousarray(
            xs.reshape(NG, G, 128, NT, BT).transpose(3, 0, 2, 1, 4)
              .reshape(NT, NG, 128, G * BT))
        in_maps.append({"xt": xr, "wt0": wt0, "wt1": wt1, "bb": bb})
    return in_maps


def kernel(x, noise, router_w, router_b, noise_w, noise_b, expert_w, expert_b,
           _trace=False):
    from concourse.bass_utils import run_bass_kernel_spmd

    x = np.asarray(x, dtype=np.float32)
    nc = get_program()
    in_maps = make_in_maps(x, np.asarray(noise_w), np.asarray(noise_b),
                           np.asarray(expert_w), np.asarray(expert_b))
    res = run_bass_kernel_spmd(nc, in_maps, core_ids=list(range(NCORES)),
                               trace=_trace)
    out = np.concatenate([r["out"] for r in res.results], axis=0)
    if _trace:
        kernel.last_results = res
    return out


# revision 8
# speedup vs baseline: 2.0191x; 1.0378x over previous
"""MoE logistic regression kernel for 8 Trainium2 NeuronCores.

Math (after dead-code elimination of the reference's unused router path):
    noise_logits = x @ noise_w.T + noise_b            # [B, E]
    top8 = top_k(noise_logits, 8)
    gates = softmax over the top-8 entries (others 0)
    expert = sigmoid(x @ expert_w.T + expert_b)       # [B, E]
    out[b] = sum_e gates[b,e] * expert[b,e]           # [B, 1]

Sharding: batch split 8 ways (2048 rows/core); weights replicated.

Implementation notes:
- Single-pass fp16 matmul (x, w rounded on host). Logit error ~2.3e-4
  flips the 8th/9th expert on ~24/16384 rows; end-to-end l2 rel err
  ~1.2e-3 vs the 2e-2 gate, at half the DMA and a third of the PE work
  of an fp16 hi/lo split.
- Batch-tile-major stream: each 512-row tile's full contraction arrives
  while the previous tile's top-8/softmax/sigmoid epilogue runs on
  ACT/DVE. The last tile's DMA groups taper (...,4,2,1,1 k-chunks) so
  almost no matmul work remains after the last byte lands.
- The epilogue never uses the ACT sigmoid table: sigmoid comes from
  exp(-z) + DVE 1/(1+e), and softmax skips the max-shift (logits are
  bounded ~|4|), so every ACT op lives in the one exp_and_others
  function set -- no mid-stream LoadActFuncSet (1.3us each).
- Top-8 gating via Max8 + a fused (v >= v8) mask * exp(v) with
  accumulated row sum (scalar_tensor_tensor), and the sigmoid divide
  fused into tensor_tensor_reduce.
"""

import sys

import numpy as np

if "/opt/trn_rl_repo" not in sys.path:
    sys.path.insert(0, "/opt/trn_rl_repo")

B, D, E, TOPK, NCORES = 16384, 4096, 64, 8, 8
BC = B // NCORES      # batch rows per core
BT = 512              # batch tile (one PSUM bank of fp32)
NT = BC // BT         # batch tiles per core
NK = D // 128         # contraction chunks
# DMA grouping (k-chunks per transfer); last tile tapers to shrink the tail
GROUPS = [[8, 8, 8, 8]] * (NT - 1) + [[8, 8, 8, 4, 2, 1, 1]]

_cached = {}


def _build_program():
    import concourse.bass as bass
    import concourse.tile as tile
    from concourse import bacc, mybir
    from concourse.masks import make_identity

    f32 = mybir.dt.float32
    f16 = mybir.dt.float16
    act = mybir.ActivationFunctionType
    alu = mybir.AluOpType

    nc = bacc.Bacc("TRN2", target_bir_lowering=False, debug=False)
    # x fp16, [t, k, p, b]: contraction row k*128+p, batch col t*BT+b.
    xt = nc.dram_tensor("xt", [NT, NK, 128, BT], f16, kind="ExternalInput").ap()
    # weights: partition p holds, for each k, the 128 output cols of
    # contraction row k*128+p. Split so group 0 can load first.
    wt0 = nc.dram_tensor("wt0", [128, 8 * 128], f16, kind="ExternalInput").ap()
    wt1 = nc.dram_tensor("wt1", [128, (NK - 8) * 128], f16,
                         kind="ExternalInput").ap()
    bb = nc.dram_tensor("bb", [128, 1], f32, kind="ExternalInput").ap()
    out = nc.dram_tensor("out", [BC, 1], f32, kind="ExternalOutput").ap()
    out_v = out.rearrange("(t j p) o -> t p (j o)", t=NT, j=4, p=128)

    with tile.TileContext(nc) as tc:
        with (
            tc.tile_pool(name="consts", bufs=1) as consts,
            tc.tile_pool(name="xpool", bufs=6) as xpool,
            tc.tile_pool(name="eppool", bufs=4) as eppool,
            tc.tile_pool(name="small", bufs=2) as small,
            tc.tile_pool(name="tvp", bufs=8) as tvp,
            tc.tile_pool(name="psacc", bufs=1, space=bass.MemorySpace.PSUM) as psacc,
            tc.tile_pool(name="pstr", bufs=2, space=bass.MemorySpace.PSUM) as pstr,
        ):
            # ---- constants ----
            w0_sb = consts.tile([128, 8, 128], f16)
            nc.sync.dma_start(out=w0_sb,
                              in_=wt0.rearrange("p (g m) -> p g m", g=8))
            w1_sb = consts.tile([128, NK - 8, 128], f16)
            nc.sync.dma_start(out=w1_sb,
                              in_=wt1.rearrange("p (g m) -> p g m", g=NK - 8))
            bb_sb = consts.tile([128, 1], f32)
            nc.gpsimd.dma_start(out=bb_sb, in_=bb)
            ident = consts.tile([128, 128], f32)
            make_identity(nc, ident)
            # warm the ACT exp_and_others table during the DMA phase; every
            # later ACT op (Identity/Copy/Exp) stays in this one set.
            warm = consts.tile([1, 1], f32)
            nc.vector.memset(warm, 0.0)
            nc.scalar.add(warm, warm, bb_sb[0:1, :])
            nc.scalar.activation(warm, warm, func=act.Exp)

            accs = [psacc.tile([128, BT], f32, tag=f"acc{t}", name=f"acc{t}")
                    for t in range(NT)]

            for t in range(NT):
                # ---- stream tile t's contraction, accumulate logits.T ----
                # acc[0:64,:] = noise logits.T, acc[64:128,:] = expert
                # logits.T (both pre-bias)
                k0 = 0
                for gsz in GROUPS[t]:
                    xk = xpool.tile([128, gsz, BT], f16, tag=f"xk{gsz}")
                    nc.sync.dma_start(
                        out=xk,
                        in_=xt[t, k0:k0 + gsz].rearrange("g p b -> p g b"))
                    for g in range(gsz):
                        k = k0 + g
                        w = w0_sb[:, k, :] if k < 8 else w1_sb[:, k - 8, :]
                        nc.tensor.matmul(accs[t], lhsT=w, rhs=xk[:, g, :],
                                         start=(k == 0), stop=(k == NK - 1))
                    k0 += gsz

                # ---- epilogue for tile t (overlaps tile t+1's stream) ----
                # bias-add both halves PSUM->SBUF (base-0 tiles for the PE
                # transposes)
                noiseT = eppool.tile([64, BT], f32, tag="noiseT")
                nc.scalar.add(noiseT, accs[t][0:64, :], bb_sb[0:64, :])
                expT = eppool.tile([64, BT], f32, tag="expT")
                nc.scalar.add(expT, accs[t][64:128, :], bb_sb[64:128, :])
                # transpose to batch-major: [128 batch, j | 4+j, 64]
                ps_ne = pstr.tile([128, 8, 64], f32, tag="ps_ne",
                                  name=f"ps_ne{t}")
                for j in range(4):
                    nc.tensor.transpose(ps_ne[:, j, :],
                                        noiseT[:, j * 128:(j + 1) * 128],
                                        ident[0:64, 0:64])
                    nc.tensor.transpose(ps_ne[:, 4 + j, :],
                                        expT[:, j * 128:(j + 1) * 128],
                                        ident[0:64, 0:64])
                # softmax numerator without max-shift (|logit| <~ 4)
                e_all = small.tile([128, 4, 64], f32, tag="e_all")
                nc.scalar.activation(e_all, ps_ne[:, 0:4, :], func=act.Exp)
                # sigmoid, part 1: exp(-z); 1/(1+e) folded into the DVE ops
                eex = small.tile([128, 4, 64], f32, tag="eex")
                nc.scalar.activation(eex, ps_ne[:, 4:8, :], func=act.Exp,
                                     scale=-1.0)
                tvs = []
                for j in range(4):
                    tv = tvp.tile([128, 8], f32, tag="tv", name=f"tv{t}_{j}")
                    nc.vector.max(tv, ps_ne[:, j, :])     # top-8, descending
                    tvs.append(tv)
                # g = exp(v) where v >= v8 else 0; zsum = row sum of g
                gts = small.tile([128, 4, 64], f32, tag="gts")
                zsum = small.tile([128, 4], f32, tag="zsum")
                for j in range(4):
                    nc.vector.scalar_tensor_tensor(
                        out=gts[:, j, :], in0=ps_ne[:, j, :],
                        scalar=tvs[j][:, 7:8], in1=e_all[:, j, :],
                        op0=alu.is_ge, op1=alu.mult,
                        accum_out=zsum[:, j:j + 1])
                den = small.tile([128, 4, 64], f32, tag="den")
                nc.vector.tensor_scalar_add(den, eex, 1.0)
                sig = small.tile([128, 4, 64], f32, tag="sig")
                nc.vector.reciprocal(sig, den)
                # s4 = sum_e g*sigmoid = sum_e g/(1+exp(-z))
                scr = small.tile([128, 4, 64], f32, tag="scr")
                s4 = small.tile([128, 4], f32, tag="s4")
                for j in range(4):
                    nc.vector.scalar_tensor_tensor(
                        out=scr[:, j, :], in0=gts[:, j, :], scalar=1.0,
                        in1=sig[:, j, :], op0=alu.mult, op1=alu.mult,
                        accum_out=s4[:, j:j + 1])
                rz = small.tile([128, 4], f32, tag="rz")
                nc.vector.reciprocal(rz, zsum)
                fin4 = small.tile([128, 4], f32, tag="fin4")
                nc.vector.tensor_mul(fin4, s4, rz)
                nc.sync.dma_start(out=out_v[t], in_=fin4)

    nc.compile()
    return nc


def get_program():
    if "prog" not in _cached:
        _cached["prog"] = _build_program()
    return _cached["prog"]


def make_in_maps(x, noise_w, noise_b, expert_w, expert_b):
    """Host-side sharding: per-core transposed fp16 x slice + weights."""
    w_comb = np.concatenate([noise_w, expert_w], axis=0).astype(np.float32)  # [128, D]
    wt32 = np.ascontiguousarray(w_comb.T).astype(np.float16)                 # [D, 128]
    # partition p holds [nk, 128] for contraction rows nk*128+p
    wt = np.ascontiguousarray(
        wt32.reshape(NK, 128, 128).transpose(1, 0, 2).reshape(128, -1))
    wt0 = np.ascontiguousarray(wt[:, :8 * 128])
    wt1 = np.ascontiguousarray(wt[:, 8 * 128:])
    bb = np.concatenate([noise_b, expert_b]).astype(np.float32).reshape(128, 1)
    in_maps = []
    for c in range(NCORES):
        xs = np.ascontiguousarray(x[c * BC:(c + 1) * BC, :].T).astype(np.float16)
        # [D, BC] -> [NT, NK, 128, BT]: [k*128+p, t*BT+b] -> [t, k, p, b]
        xr = np.ascontiguousarray(
            xs.reshape(NK, 128, NT, BT).transpose(2, 0, 1, 3))
        in_maps.append({"xt": xr, "wt0": wt0, "wt1": wt1, "bb": bb})
    return in_maps


def kernel(x, noise, router_w, router_b, noise_w, noise_b, expert_w, expert_b,
           _trace=False):
    from concourse.bass_utils import run_bass_kernel_spmd

    x = np.asarray(x, dtype=np.float32)
    nc = get_program()
    in_maps = make_in_maps(x, np.asarray(noise_w), np.asarray(noise_b),
                           np.asarray(expert_w), np.asarray(expert_b))
    res = run_bass_kernel_spmd(nc, in_maps, core_ids=list(range(NCORES)),
                               trace=_trace)
    out = np.concatenate([r["out"] for r in res.results], axis=0)
    if _trace:
        kernel.last_results = res
    return out


# revision 9
# speedup vs baseline: 2.1500x; 1.0648x over previous
"""MoE logistic regression kernel for 8 Trainium2 NeuronCores.

Math (after dead-code elimination of the reference's unused router path):
    noise_logits = x @ noise_w.T + noise_b            # [B, E]
    top8 = top_k(noise_logits, 8)
    gates = softmax over the top-8 entries (others 0)
    expert = sigmoid(x @ expert_w.T + expert_b)       # [B, E]
    out[b] = sum_e gates[b,e] * expert[b,e]           # [B, 1]

Sharding: batch split 8 ways (2048 rows/core); weights replicated.

Implementation notes:
- Single-pass fp16 matmul (x, w rounded on host). Logit error ~2.3e-4
  flips the 8th/9th expert on ~24/16384 rows; end-to-end l2 rel err
  ~1.2e-3 vs the 2e-2 gate, at half the DMA and a third of the PE work
  of an fp16 hi/lo split.
- Batch-tile-major stream: each 512-row tile's full contraction arrives
  while the previous tile's top-8/softmax/sigmoid epilogue runs on
  ACT/DVE. The last tile's DMA groups taper (...,4,2,1,1 k-chunks) so
  almost no matmul work remains after the last byte lands.
- The epilogue never uses the ACT sigmoid table: sigmoid comes from
  exp(-z) + DVE 1/(1+e), and softmax skips the max-shift (logits are
  bounded ~|4|), so every ACT op lives in the one exp_and_others
  function set -- no mid-stream LoadActFuncSet (1.3us each).
- Top-8 gating via Max8 + a fused (v >= v8) mask * exp(v) with
  accumulated row sum (scalar_tensor_tensor), and the sigmoid divide
  fused into tensor_tensor_reduce.
"""

import sys

import numpy as np

if "/opt/trn_rl_repo" not in sys.path:
    sys.path.insert(0, "/opt/trn_rl_repo")

B, D, E, TOPK, NCORES = 16384, 4096, 64, 8, 8
BC = B // NCORES      # batch rows per core
BT = 512              # batch tile (one PSUM bank of fp32)
NT = BC // BT         # batch tiles per core
NK = D // 128         # contraction chunks
# DMA grouping (k-chunks per transfer); last tile tapers to shrink the tail
GROUPS = [[8, 8, 8, 8]] * (NT - 1) + [[8, 8, 8, 5, 2, 1]]

_cached = {}


def _build_program():
    import concourse.bass as bass
    import concourse.tile as tile
    from concourse import bacc, mybir
    from concourse.masks import make_identity

    f32 = mybir.dt.float32
    f16 = mybir.dt.float16
    act = mybir.ActivationFunctionType
    alu = mybir.AluOpType

    nc = bacc.Bacc("TRN2", target_bir_lowering=False, debug=False)
    # x fp16, [t, k, p, b]: contraction row k*128+p, batch col t*BT+b.
    xt = nc.dram_tensor("xt", [NT, NK, 128, BT], f16, kind="ExternalInput").ap()
    # weights: partition p holds, for each k, the 128 output cols of
    # contraction row k*128+p. Split so group 0 can load first.
    wt0 = nc.dram_tensor("wt0", [128, 8 * 128], f16, kind="ExternalInput").ap()
    wt1 = nc.dram_tensor("wt1", [128, (NK - 8) * 128], f16,
                         kind="ExternalInput").ap()
    bb = nc.dram_tensor("bb", [128, 1], f32, kind="ExternalInput").ap()
    out = nc.dram_tensor("out", [BC, 1], f32, kind="ExternalOutput").ap()
    out_v = out.rearrange("(t j p) o -> t p (j o)", t=NT, j=4, p=128)

    with tile.TileContext(nc) as tc:
        with (
            tc.tile_pool(name="consts", bufs=1) as consts,
            tc.tile_pool(name="xpool", bufs=6) as xpool,
            tc.tile_pool(name="eppool", bufs=4) as eppool,
            tc.tile_pool(name="small", bufs=2) as small,
            tc.tile_pool(name="tvp", bufs=8) as tvp,
            tc.tile_pool(name="psacc", bufs=1, space=bass.MemorySpace.PSUM) as psacc,
            tc.tile_pool(name="pstr", bufs=2, space=bass.MemorySpace.PSUM) as pstr,
        ):
            # ---- constants ----
            w0_sb = consts.tile([128, 8, 128], f16)
            nc.sync.dma_start(out=w0_sb,
                              in_=wt0.rearrange("p (g m) -> p g m", g=8))
            w1_sb = consts.tile([128, NK - 8, 128], f16)
            nc.sync.dma_start(out=w1_sb,
                              in_=wt1.rearrange("p (g m) -> p g m", g=NK - 8))
            bb_sb = consts.tile([128, 1], f32)
            nc.gpsimd.dma_start(out=bb_sb, in_=bb)
            ident = consts.tile([128, 128], f32)
            make_identity(nc, ident)
            # warm the ACT exp_and_others table during the DMA phase; every
            # later ACT op (Identity/Copy/Exp) stays in this one set.
            warm = consts.tile([1, 1], f32)
            nc.vector.memset(warm, 0.0)
            nc.scalar.add(warm, warm, bb_sb[0:1, :])
            nc.scalar.activation(warm, warm, func=act.Exp)

            accs = [psacc.tile([128, BT], f32, tag=f"acc{t}", name=f"acc{t}")
                    for t in range(NT)]

            for t in range(NT):
                # ---- stream tile t's contraction, accumulate logits.T ----
                # acc[0:64,:] = noise logits.T, acc[64:128,:] = expert
                # logits.T (both pre-bias)
                k0 = 0
                for gsz in GROUPS[t]:
                    xk = xpool.tile([128, gsz, BT], f16, tag=f"xk{gsz}")
                    nc.sync.dma_start(
                        out=xk,
                        in_=xt[t, k0:k0 + gsz].rearrange("g p b -> p g b"))
                    for g in range(gsz):
                        k = k0 + g
                        w = w0_sb[:, k, :] if k < 8 else w1_sb[:, k - 8, :]
                        nc.tensor.matmul(accs[t], lhsT=w, rhs=xk[:, g, :],
                                         start=(k == 0), stop=(k == NK - 1))
                    k0 += gsz

                # ---- epilogue for tile t (overlaps tile t+1's stream) ----
                # bias-add both halves PSUM->SBUF (base-0 tiles for the PE
                # transposes)
                noiseT = eppool.tile([64, BT], f32, tag="noiseT")
                nc.scalar.add(noiseT, accs[t][0:64, :], bb_sb[0:64, :])
                expT = eppool.tile([64, BT], f32, tag="expT")
                nc.vector.tensor_scalar_add(expT, accs[t][64:128, :],
                                            bb_sb[64:128, :])
                # transpose to batch-major: [128 batch, j | 4+j, 64];
                # noise half first so e_all/Max start as early as possible
                ps_ne = pstr.tile([128, 8, 64], f32, tag="ps_ne",
                                  name=f"ps_ne{t}")
                for j in range(4):
                    nc.tensor.transpose(ps_ne[:, j, :],
                                        noiseT[:, j * 128:(j + 1) * 128],
                                        ident[0:64, 0:64])
                for j in range(4):
                    nc.tensor.transpose(ps_ne[:, 4 + j, :],
                                        expT[:, j * 128:(j + 1) * 128],
                                        ident[0:64, 0:64])
                # softmax numerator without max-shift (|logit| <~ 4)
                e_all = small.tile([128, 4, 64], f32, tag="e_all")
                nc.scalar.activation(e_all, ps_ne[:, 0:4, :], func=act.Exp)
                # sigmoid, part 1: exp(-z); 1/(1+e) folded into the DVE ops
                eex = small.tile([128, 4, 64], f32, tag="eex")
                nc.scalar.activation(eex, ps_ne[:, 4:8, :], func=act.Exp,
                                     scale=-1.0)
                tvs = []
                for j in range(4):
                    tv = tvp.tile([128, 8], f32, tag="tv", name=f"tv{t}_{j}")
                    nc.vector.max(tv, ps_ne[:, j, :])     # top-8, descending
                    tvs.append(tv)
                # g = exp(v) where v >= v8 else 0; zsum = row sum of g
                gts = small.tile([128, 4, 64], f32, tag="gts")
                zsum = small.tile([128, 4], f32, tag="zsum")
                for j in range(4):
                    nc.vector.scalar_tensor_tensor(
                        out=gts[:, j, :], in0=ps_ne[:, j, :],
                        scalar=tvs[j][:, 7:8], in1=e_all[:, j, :],
                        op0=alu.is_ge, op1=alu.mult,
                        accum_out=zsum[:, j:j + 1])
                den = small.tile([128, 4, 64], f32, tag="den")
                nc.vector.tensor_scalar_add(den, eex, 1.0)
                sig = small.tile([128, 4, 64], f32, tag="sig")
                nc.vector.reciprocal(sig, den)
                # s4 = sum_e g*sigmoid = sum_e g/(1+exp(-z))
                scr = small.tile([128, 4, 64], f32, tag="scr")
                s4 = small.tile([128, 4], f32, tag="s4")
                for j in range(4):
                    nc.vector.scalar_tensor_tensor(
                        out=scr[:, j, :], in0=gts[:, j, :], scalar=1.0,
                        in1=sig[:, j, :], op0=alu.mult, op1=alu.mult,
                        accum_out=s4[:, j:j + 1])
                rz = small.tile([128, 4], f32, tag="rz")
                nc.vector.reciprocal(rz, zsum)
                fin4 = small.tile([128, 4], f32, tag="fin4")
                nc.vector.tensor_mul(fin4, s4, rz)
                nc.gpsimd.dma_start(out=out_v[t], in_=fin4)

    nc.compile()
    return nc


def get_program():
    if "prog" not in _cached:
        _cached["prog"] = _build_program()
    return _cached["prog"]


def make_in_maps(x, noise_w, noise_b, expert_w, expert_b):
    """Host-side sharding: per-core transposed fp16 x slice + weights."""
    w_comb = np.concatenate([noise_w, expert_w], axis=0).astype(np.float32)  # [128, D]
    wt32 = np.ascontiguousarray(w_comb.T).astype(np.float16)                 # [D, 128]
    # partition p holds [nk, 128] for contraction rows nk*128+p
    wt = np.ascontiguousarray(
        wt32.reshape(NK, 128, 128).transpose(1, 0, 2).reshape(128, -1))
    wt0 = np.ascontiguousarray(wt[:, :8 * 128])
    wt1 = np.ascontiguousarray(wt[:, 8 * 128:])
    bb = np.concatenate([noise_b, expert_b]).astype(np.float32).reshape(128, 1)
    in_maps = []
    for c in range(NCORES):
        xs = np.ascontiguousarray(x[c * BC:(c + 1) * BC, :].T).astype(np.float16)
        # [D, BC] -> [NT, NK, 128, BT]: [k*128+p, t*BT+b] -> [t, k, p, b]
        xr = np.ascontiguousarray(
            xs.reshape(NK, 128, NT, BT).transpose(2, 0, 1, 3))
        in_maps.append({"xt": xr, "wt0": wt0, "wt1": wt1, "bb": bb})
    return in_maps


def kernel(x, noise, router_w, router_b, noise_w, noise_b, expert_w, expert_b,
           _trace=False):
    from concourse.bass_utils import run_bass_kernel_spmd

    x = np.asarray(x, dtype=np.float32)
    nc = get_program()
    in_maps = make_in_maps(x, np.asarray(noise_w), np.asarray(noise_b),
                           np.asarray(expert_w), np.asarray(expert_b))
    res = run_bass_kernel_spmd(nc, in_maps, core_ids=list(range(NCORES)),
                               trace=_trace)
    out = np.concatenate([r["out"] for r in res.results], axis=0)
    if _trace:
        kernel.last_results = res
    return out


# revision 11
# speedup vs baseline: 2.3213x; 1.0797x over previous
"""MoE logistic regression kernel for 8 Trainium2 NeuronCores.

Math (after dead-code elimination of the reference's unused router path):
    noise_logits = x @ noise_w.T + noise_b            # [B, E]
    top8 = top_k(noise_logits, 8)
    gates = softmax over the top-8 entries (others 0)
    expert = sigmoid(x @ expert_w.T + expert_b)       # [B, E]
    out[b] = sum_e gates[b,e] * expert[b,e]           # [B, 1]

Sharding: batch split 8 ways (2048 rows/core); weights replicated.

Implementation notes:
- Single-pass fp16 matmul (x, w rounded on host). Logit error ~2.3e-4
  flips the 8th/9th expert on ~24/16384 rows; end-to-end l2 rel err
  ~1.2e-3 vs the 2e-2 gate, at half the DMA and a third of the PE work
  of an fp16 hi/lo split. The x stream is the roofline: ~46.6us of
  gapless DMA at the HBM limit.
- Batch-tile-major stream: each tile's full contraction arrives while
  the previous tile's epilogue runs on ACT/DVE. Tile widths taper
  (512,512,512,256,128,128) and the last tile's DMA groups taper too,
  so the serial tail after the last byte is one short epilogue.
- The epilogue never uses the ACT sigmoid table: sigmoid comes from
  exp(-z) + DVE 1/(1+e), and softmax skips the max-shift (logits are
  bounded ~|4|), so every ACT op stays in the one exp_and_others
  function set -- no mid-stream LoadActFuncSet (1.3us each).
- Top-8 gating via DVE Max8 + fused (e >= e8) mask * e with accumulated
  row sum (scalar_tensor_tensor), all on the SBUF exp(v) copy --
  exp is monotone so selection is identical, and avoiding a second
  PSUM reader dodges cross-engine read serialization.
- Per-tile outputs DMA straight from the [128, j] result (no final
  transpose); early tiles go via the idle gpsimd SWDGE path so they
  never head-of-line-block the x stream on the SP HWDGE queue.
"""

import sys

import numpy as np

if "/opt/trn_rl_repo" not in sys.path:
    sys.path.insert(0, "/opt/trn_rl_repo")

B, D, E, TOPK, NCORES = 16384, 4096, 64, 8, 8
BC = B // NCORES      # batch rows per core
NK = D // 128         # contraction chunks
TILES = [512, 512, 512, 256, 128, 128]          # batch tile widths
OFFS = [sum(TILES[:i]) for i in range(len(TILES))]
assert sum(TILES) == BC
# DMA grouping in k-chunks per tile; the final tile tapers so almost no
# matmul work remains after the last byte lands
GROUPS = [[8, 8, 8, 8]] * 5 + [[16, 8, 4, 2, 2]]

_cached = {}


def _build_program():
    import concourse.bass as bass
    import concourse.tile as tile
    from concourse import bacc, mybir
    from concourse.masks import make_identity

    f32 = mybir.dt.float32
    f16 = mybir.dt.float16
    act = mybir.ActivationFunctionType
    alu = mybir.AluOpType

    nc = bacc.Bacc("TRN2", target_bir_lowering=False, debug=False)
    # x fp16, per-tile partition-major blocks concatenated flat:
    # tile t occupies [128, NK, bt] at element offset 128*NK*OFFS[t], so
    # every group DMA is one contiguous gsz*bt*2-byte run per partition.
    xt = nc.dram_tensor("xt", [NK * 128 * BC], f16, kind="ExternalInput").ap()
    wt0 = nc.dram_tensor("wt0", [128, 8 * 128], f16, kind="ExternalInput").ap()
    wt1 = nc.dram_tensor("wt1", [128, (NK - 8) * 128], f16,
                         kind="ExternalInput").ap()
    bb = nc.dram_tensor("bb", [128, 1], f32, kind="ExternalInput").ap()
    out = nc.dram_tensor("out", [BC, 1], f32, kind="ExternalOutput").ap()

    with tile.TileContext(nc) as tc:
        with (
            tc.tile_pool(name="consts", bufs=1) as consts,
            tc.tile_pool(name="xpool", bufs=6) as xpool,
            tc.tile_pool(name="eppool", bufs=4) as eppool,
            tc.tile_pool(name="small", bufs=2) as small,
            tc.tile_pool(name="tvp", bufs=8) as tvp,
            tc.tile_pool(name="psacc", bufs=1, space=bass.MemorySpace.PSUM) as psacc,
            tc.tile_pool(name="pstr", bufs=2, space=bass.MemorySpace.PSUM) as pstr,
        ):
            # ---- constants ----
            w0_sb = consts.tile([128, 8, 128], f16)
            nc.sync.dma_start(out=w0_sb,
                              in_=wt0.rearrange("p (g m) -> p g m", g=8))
            w1_sb = consts.tile([128, NK - 8, 128], f16)
            nc.sync.dma_start(out=w1_sb,
                              in_=wt1.rearrange("p (g m) -> p g m", g=NK - 8))
            bb_sb = consts.tile([128, 1], f32)
            nc.gpsimd.dma_start(out=bb_sb, in_=bb)
            ident = consts.tile([128, 128], f32)
            make_identity(nc, ident)
            # warm the ACT exp_and_others table during the DMA phase; every
            # later ACT op (Identity/Copy/Exp) stays in this one set.
            warm = consts.tile([1, 1], f32)
            nc.vector.memset(warm, 0.0)
            nc.scalar.add(warm, warm, bb_sb[0:1, :])
            nc.scalar.activation(warm, warm, func=act.Exp)

            accs = [psacc.tile([128, 512], f32, tag=f"acc{t}", name=f"acc{t}")
                    for t in range(len(TILES))]

            for t, bt in enumerate(TILES):
                njs = bt // 128
                off = OFFS[t]
                acc = accs[t][:, 0:bt]
                # ---- stream tile t's contraction, accumulate logits.T ----
                # acc[0:64,:] = noise logits.T, acc[64:128,:] = expert
                # logits.T (both pre-bias)
                base = 128 * NK * off
                xtile = xt[base:base + 128 * NK * bt].rearrange(
                    "(p k b) -> p k b", p=128, k=NK)
                k0 = 0
                for gsz in GROUPS[t]:
                    xk = xpool.tile([128, gsz, bt], f16, tag=f"xk{bt}_{gsz}")
                    nc.sync.dma_start(out=xk, in_=xtile[:, k0:k0 + gsz, :])
                    for g in range(gsz):
                        k = k0 + g
                        w = w0_sb[:, k, :] if k < 8 else w1_sb[:, k - 8, :]
                        nc.tensor.matmul(acc, lhsT=w, rhs=xk[:, g, :],
                                         start=(k == 0), stop=(k == NK - 1))
                    k0 += gsz

                # ---- epilogue for tile t (overlaps tile t+1's stream) ----
                # bias-add both halves PSUM->SBUF: noise on ACT, expert on
                # DVE, in parallel
                noiseT = eppool.tile([64, bt], f32, tag=f"nT{bt}")
                nc.scalar.add(noiseT, accs[t][0:64, 0:bt], bb_sb[0:64, :])
                expT = eppool.tile([64, bt], f32, tag=f"eT{bt}")
                nc.vector.tensor_scalar_add(expT, accs[t][64:128, 0:bt],
                                            bb_sb[64:128, :])
                # transpose to batch-major: [128 batch, j | 4+j, 64];
                # noise half first so e_all starts as early as possible
                ps_ne = pstr.tile([128, 8, 64], f32, tag="ps_ne",
                                  name=f"ps_ne{t}")
                for j in range(njs):
                    nc.tensor.transpose(ps_ne[:, j, :],
                                        noiseT[:, j * 128:(j + 1) * 128],
                                        ident[0:64, 0:64])
                for j in range(njs):
                    nc.tensor.transpose(ps_ne[:, 4 + j, :],
                                        expT[:, j * 128:(j + 1) * 128],
                                        ident[0:64, 0:64])
                # softmax numerator without max-shift (|logit| <~ 4); the
                # only readers of ps_ne are the two ACT exps, so the DVE
                # chain below runs entirely from SBUF
                e_all = small.tile([128, 4, 64], f32, tag="e_all")
                nc.scalar.activation(e_all[:, 0:njs, :], ps_ne[:, 0:njs, :],
                                     func=act.Exp)
                # sigmoid, part 1: exp(-z); 1/(1+e) via DVE
                eex = small.tile([128, 4, 64], f32, tag="eex")
                nc.scalar.activation(eex[:, 0:njs, :], ps_ne[:, 4:4 + njs, :],
                                     func=act.Exp, scale=-1.0)
                # top-8 on exp(v) (monotone => same selection as on v)
                tvs = []
                for j in range(njs):
                    tv = tvp.tile([128, 8], f32, tag="tv", name=f"tv{t}_{j}")
                    nc.vector.max(tv, e_all[:, j, :])
                    tvs.append(tv)
                # g = e where e >= e8 else 0; zsum = row sum of g
                gts = small.tile([128, 4, 64], f32, tag="gts")
                zsum = small.tile([128, 4], f32, tag="zsum")
                for j in range(njs):
                    nc.vector.scalar_tensor_tensor(
                        out=gts[:, j, :], in0=e_all[:, j, :],
                        scalar=tvs[j][:, 7:8], in1=e_all[:, j, :],
                        op0=alu.is_ge, op1=alu.mult,
                        accum_out=zsum[:, j:j + 1])
                den = small.tile([128, 4, 64], f32, tag="den")
                nc.vector.tensor_scalar_add(den[:, 0:njs, :], eex[:, 0:njs, :],
                                            1.0)
                sig = small.tile([128, 4, 64], f32, tag="sig")
                nc.vector.reciprocal(sig[:, 0:njs, :], den[:, 0:njs, :])
                # s4 = sum_e g*sigmoid
                scr = small.tile([128, 4, 64], f32, tag="scr")
                s4 = small.tile([128, 4], f32, tag="s4")
                for j in range(njs):
                    nc.vector.scalar_tensor_tensor(
                        out=scr[:, j, :], in0=gts[:, j, :], scalar=1.0,
                        in1=sig[:, j, :], op0=alu.mult, op1=alu.mult,
                        accum_out=s4[:, j:j + 1])
                rz = small.tile([128, 4], f32, tag="rz")
                nc.vector.reciprocal(rz[:, 0:njs], zsum[:, 0:njs])
                fin = small.tile([128, 4], f32, tag="fin")
                nc.vector.tensor_mul(fin[:, 0:njs], s4[:, 0:njs],
                                     rz[:, 0:njs])
                out_t = out[off:off + bt, :].rearrange(
                    "(j p) o -> p (j o)", j=njs, p=128)
                eng = nc.sync if t == len(TILES) - 1 else nc.gpsimd
                eng.dma_start(out=out_t, in_=fin[:, 0:njs])

    nc.compile()
    return nc


def get_program():
    if "prog" not in _cached:
        _cached["prog"] = _build_program()
    return _cached["prog"]


def make_in_maps(x, noise_w, noise_b, expert_w, expert_b):
    """Host-side sharding: per-core transposed fp16 x slice + weights."""
    w_comb = np.concatenate([noise_w, expert_w], axis=0).astype(np.float32)  # [128, D]
    wt32 = np.ascontiguousarray(w_comb.T).astype(np.float16)                 # [D, 128]
    # partition p holds [nk, 128] for contraction rows nk*128+p
    wt = np.ascontiguousarray(
        wt32.reshape(NK, 128, 128).transpose(1, 0, 2).reshape(128, -1))
    wt0 = np.ascontiguousarray(wt[:, :8 * 128])
    wt1 = np.ascontiguousarray(wt[:, 8 * 128:])
    bb = np.concatenate([noise_b, expert_b]).astype(np.float32).reshape(128, 1)
    in_maps = []
    for c in range(NCORES):
        xs = np.ascontiguousarray(x[c * BC:(c + 1) * BC, :].T).astype(np.float16)
        # per tile: [D, bt] -> [128, NK, bt], concatenated flat
        blocks = []
        for t, bt in enumerate(TILES):
            blk = xs[:, OFFS[t]:OFFS[t] + bt].reshape(NK, 128, bt)
            blocks.append(blk.transpose(1, 0, 2).reshape(-1))
        xr = np.ascontiguousarray(np.concatenate(blocks))
        in_maps.append({"xt": xr, "wt0": wt0, "wt1": wt1, "bb": bb})
    return in_maps


def kernel(x, noise, router_w, router_b, noise_w, noise_b, expert_w, expert_b,
           _trace=False):
    from concourse.bass_utils import run_bass_kernel_spmd

    x = np.asarray(x, dtype=np.float32)
    nc = get_program()
    in_maps = make_in_maps(x, np.asarray(noise_w), np.asarray(noise_b),
                           np.asarray(expert_w), np.asarray(expert_b))
    res = run_bass_kernel_spmd(nc, in_maps, core_ids=list(range(NCORES)),
                               trace=_trace)
    out = np.concatenate([r["out"] for r in res.results], axis=0)
    if _trace:
        kernel.last_results = res
    return out


# revision 12
# speedup vs baseline: 2.3302x; 1.0038x over previous
"""MoE logistic regression kernel for 8 Trainium2 NeuronCores.

Math (after dead-code elimination of the reference's unused router path):
    noise_logits = x @ noise_w.T + noise_b            # [B, E]
    top8 = top_k(noise_logits, 8)
    gates = softmax over the top-8 entries (others 0)
    expert = sigmoid(x @ expert_w.T + expert_b)       # [B, E]
    out[b] = sum_e gates[b,e] * expert[b,e]           # [B, 1]

Sharding: batch split 8 ways (2048 rows/core); weights replicated.

Implementation notes:
- Single-pass fp16 matmul (x, w rounded on host). Logit error ~2.3e-4
  flips the 8th/9th expert on ~24/16384 rows; end-to-end l2 rel err
  ~1.2e-3 vs the 2e-2 gate, at half the DMA and a third of the PE work
  of an fp16 hi/lo split. The x stream is the roofline: ~46.6us of
  gapless DMA at the HBM limit.
- Batch-tile-major stream: each tile's full contraction arrives while
  the previous tile's epilogue runs on ACT/DVE. Tile widths taper
  (512,512,512,256,128,128) and the last tile's DMA groups taper too,
  so the serial tail after the last byte is one short epilogue.
- The epilogue never uses the ACT sigmoid table: sigmoid comes from
  exp(-z) + DVE 1/(1+e), and softmax skips the max-shift (logits are
  bounded ~|4|), so every ACT op stays in the one exp_and_others
  function set -- no mid-stream LoadActFuncSet (1.3us each).
- Top-8 gating via DVE Max8 + fused (e >= e8) mask * e with accumulated
  row sum (scalar_tensor_tensor), all on the SBUF exp(v) copy --
  exp is monotone so selection is identical, and avoiding a second
  PSUM reader dodges cross-engine read serialization.
- Per-tile outputs DMA straight from the [128, j] result (no final
  transpose); early tiles go via the idle gpsimd SWDGE path so they
  never head-of-line-block the x stream on the SP HWDGE queue.
"""

import sys

import numpy as np

if "/opt/trn_rl_repo" not in sys.path:
    sys.path.insert(0, "/opt/trn_rl_repo")

B, D, E, TOPK, NCORES = 16384, 4096, 64, 8, 8
BC = B // NCORES      # batch rows per core
NK = D // 128         # contraction chunks
TILES = [512, 512, 512, 256, 128, 128]          # batch tile widths
OFFS = [sum(TILES[:i]) for i in range(len(TILES))]
assert sum(TILES) == BC
# DMA grouping in k-chunks per tile; the final tile tapers so almost no
# matmul work remains after the last byte lands
GROUPS = [[8, 8, 8, 8]] * 5 + [[16, 8, 4, 2, 2]]

_cached = {}


def _build_program():
    import concourse.bass as bass
    import concourse.tile as tile
    from concourse import bacc, mybir
    from concourse.masks import make_identity

    f32 = mybir.dt.float32
    f16 = mybir.dt.float16
    act = mybir.ActivationFunctionType
    alu = mybir.AluOpType

    nc = bacc.Bacc("TRN2", target_bir_lowering=False, debug=False)
    # x fp16, per-tile partition-major blocks concatenated flat:
    # tile t occupies [128, NK, bt] at element offset 128*NK*OFFS[t], so
    # every group DMA is one contiguous gsz*bt*2-byte run per partition.
    xt = nc.dram_tensor("xt", [NK * 128 * BC], f16, kind="ExternalInput").ap()
    wt0 = nc.dram_tensor("wt0", [128, 8 * 128], f16, kind="ExternalInput").ap()
    wt1 = nc.dram_tensor("wt1", [128, (NK - 8) * 128], f16,
                         kind="ExternalInput").ap()
    bb = nc.dram_tensor("bb", [128, 1], f32, kind="ExternalInput").ap()
    out = nc.dram_tensor("out", [BC, 1], f32, kind="ExternalOutput").ap()

    with tile.TileContext(nc) as tc:
        with (
            tc.tile_pool(name="consts", bufs=1) as consts,
            tc.tile_pool(name="xpool", bufs=6) as xpool,
            tc.tile_pool(name="eppool", bufs=4) as eppool,
            tc.tile_pool(name="small", bufs=2) as small,
            tc.tile_pool(name="tvp", bufs=8) as tvp,
            tc.tile_pool(name="psacc", bufs=1, space=bass.MemorySpace.PSUM) as psacc,
            tc.tile_pool(name="pstr", bufs=2, space=bass.MemorySpace.PSUM) as pstr,
        ):
            # ---- constants ----
            w0_sb = consts.tile([128, 8, 128], f16)
            nc.sync.dma_start(out=w0_sb,
                              in_=wt0.rearrange("p (g m) -> p g m", g=8))
            w1_sb = consts.tile([128, NK - 8, 128], f16)
            nc.sync.dma_start(out=w1_sb,
                              in_=wt1.rearrange("p (g m) -> p g m", g=NK - 8))
            bb_sb = consts.tile([128, 1], f32)
            nc.gpsimd.dma_start(out=bb_sb, in_=bb)
            ident = consts.tile([128, 128], f32)
            make_identity(nc, ident)
            # warm the ACT exp_and_others table during the DMA phase; every
            # later ACT op (Identity/Copy/Exp) stays in this one set.
            warm = consts.tile([1, 1], f32)
            nc.vector.memset(warm, 0.0)
            nc.scalar.add(warm, warm, bb_sb[0:1, :])
            nc.scalar.activation(warm, warm, func=act.Exp)
            # tiles 0-3 stage their results here; one deferred DMA ships
            # them after the last x byte so no output transfer steals
            # mid-stream DMA time
            final_sb = consts.tile([128, 14], f32)

            accs = [psacc.tile([128, 512], f32, tag=f"acc{t}", name=f"acc{t}")
                    for t in range(len(TILES))]

            for t, bt in enumerate(TILES):
                njs = bt // 128
                off = OFFS[t]
                acc = accs[t][:, 0:bt]
                # ---- stream tile t's contraction, accumulate logits.T ----
                # acc[0:64,:] = noise logits.T, acc[64:128,:] = expert
                # logits.T (both pre-bias)
                base = 128 * NK * off
                xtile = xt[base:base + 128 * NK * bt].rearrange(
                    "(p k b) -> p k b", p=128, k=NK)
                k0 = 0
                for gsz in GROUPS[t]:
                    xk = xpool.tile([128, gsz, bt], f16, tag=f"xk{bt}_{gsz}")
                    nc.sync.dma_start(out=xk, in_=xtile[:, k0:k0 + gsz, :])
                    for g in range(gsz):
                        k = k0 + g
                        w = w0_sb[:, k, :] if k < 8 else w1_sb[:, k - 8, :]
                        nc.tensor.matmul(acc, lhsT=w, rhs=xk[:, g, :],
                                         start=(k == 0), stop=(k == NK - 1))
                    k0 += gsz

                # ---- epilogue for tile t (overlaps tile t+1's stream) ----
                # bias-add both halves PSUM->SBUF: noise on ACT, expert on
                # DVE, in parallel
                noiseT = eppool.tile([64, bt], f32, tag=f"nT{bt}")
                nc.scalar.add(noiseT, accs[t][0:64, 0:bt], bb_sb[0:64, :])
                expT = eppool.tile([64, bt], f32, tag=f"eT{bt}")
                nc.vector.tensor_scalar_add(expT, accs[t][64:128, 0:bt],
                                            bb_sb[64:128, :])
                # transpose to batch-major: [128 batch, j | 4+j, 64];
                # noise half first so e_all starts as early as possible
                ps_ne = pstr.tile([128, 8, 64], f32, tag="ps_ne",
                                  name=f"ps_ne{t}")
                for j in range(njs):
                    nc.tensor.transpose(ps_ne[:, j, :],
                                        noiseT[:, j * 128:(j + 1) * 128],
                                        ident[0:64, 0:64])
                for j in range(njs):
                    nc.tensor.transpose(ps_ne[:, 4 + j, :],
                                        expT[:, j * 128:(j + 1) * 128],
                                        ident[0:64, 0:64])
                # softmax numerator without max-shift (|logit| <~ 4); the
                # only readers of ps_ne are the two ACT exps, so the DVE
                # chain below runs entirely from SBUF
                e_all = small.tile([128, 4, 64], f32, tag="e_all")
                nc.scalar.activation(e_all[:, 0:njs, :], ps_ne[:, 0:njs, :],
                                     func=act.Exp)
                # sigmoid, part 1: exp(-z); 1/(1+e) via DVE
                eex = small.tile([128, 4, 64], f32, tag="eex")
                nc.scalar.activation(eex[:, 0:njs, :], ps_ne[:, 4:4 + njs, :],
                                     func=act.Exp, scale=-1.0)
                # top-8 on exp(v) (monotone => same selection as on v)
                tvs = []
                for j in range(njs):
                    tv = tvp.tile([128, 8], f32, tag="tv", name=f"tv{t}_{j}")
                    nc.vector.max(tv, e_all[:, j, :])
                    tvs.append(tv)
                # g = e where e >= e8 else 0; zsum = row sum of g
                gts = small.tile([128, 4, 64], f32, tag="gts")
                zsum = small.tile([128, 4], f32, tag="zsum")
                for j in range(njs):
                    nc.vector.scalar_tensor_tensor(
                        out=gts[:, j, :], in0=e_all[:, j, :],
                        scalar=tvs[j][:, 7:8], in1=e_all[:, j, :],
                        op0=alu.is_ge, op1=alu.mult,
                        accum_out=zsum[:, j:j + 1])
                den = small.tile([128, 4, 64], f32, tag="den")
                nc.vector.tensor_scalar_add(den[:, 0:njs, :], eex[:, 0:njs, :],
                                            1.0)
                sig = small.tile([128, 4, 64], f32, tag="sig")
                nc.vector.reciprocal(sig[:, 0:njs, :], den[:, 0:njs, :])
                # s4 = sum_e g*sigmoid
                scr = small.tile([128, 4, 64], f32, tag="scr")
                s4 = small.tile([128, 4], f32, tag="s4")
                for j in range(njs):
                    nc.vector.scalar_tensor_tensor(
                        out=scr[:, j, :], in0=gts[:, j, :], scalar=1.0,
                        in1=sig[:, j, :], op0=alu.mult, op1=alu.mult,
                        accum_out=s4[:, j:j + 1])
                rz = small.tile([128, 4], f32, tag="rz")
                nc.vector.reciprocal(rz[:, 0:njs], zsum[:, 0:njs])
                if t <= 3:
                    c0 = off // 128
                    nc.vector.tensor_mul(final_sb[:, c0:c0 + njs],
                                         s4[:, 0:njs], rz[:, 0:njs])
                    if t == 3:
                        nc.gpsimd.dma_start(
                            out=out[0:1792, :].rearrange(
                                "(j p) o -> p (j o)", j=14, p=128),
                            in_=final_sb)
                else:
                    fin = small.tile([128, 4], f32, tag="fin")
                    nc.vector.tensor_mul(fin[:, 0:njs], s4[:, 0:njs],
                                         rz[:, 0:njs])
                    out_t = out[off:off + bt, :].rearrange(
                        "(j p) o -> p (j o)", j=njs, p=128)
                    eng = nc.sync if t == len(TILES) - 1 else nc.gpsimd
                    eng.dma_start(out=out_t, in_=fin[:, 0:njs])

    nc.compile()
    return nc


def get_program():
    if "prog" not in _cached:
        _cached["prog"] = _build_program()
    return _cached["prog"]


def make_in_maps(x, noise_w, noise_b, expert_w, expert_b):
    """Host-side sharding: per-core transposed fp16 x slice + weights."""
    w_comb = np.concatenate([noise_w, expert_w], axis=0).astype(np.float32)  # [128, D]
    wt32 = np.ascontiguousarray(w_comb.T).astype(np.float16)                 # [D, 128]
    # partition p holds [nk, 128] for contraction rows nk*128+p
    wt = np.ascontiguousarray(
        wt32.reshape(NK, 128, 128).transpose(1, 0, 2).reshape(128, -1))
    wt0 = np.ascontiguousarray(wt[:, :8 * 128])
    wt1 = np.ascontiguousarray(wt[:, 8 * 128:])
    bb = np.concatenate([noise_b, expert_b]).astype(np.float32).reshape(128, 1)
    in_maps = []
    for c in range(NCORES):
        xs = np.ascontiguousarray(x[c * BC:(c + 1) * BC, :].T).astype(np.float16)
        # per tile: [D, bt] -> [128, NK, bt], concatenated flat
        blocks = []
        for t, bt in enumerate(TILES):
            blk = xs[:, OFFS[t]:OFFS[t] + bt].reshape(NK, 128, bt)
            blocks.append(blk.transpose(1, 0, 2).reshape(-1))
        xr = np.ascontiguousarray(np.concatenate(blocks))
        in_maps.append({"xt": xr, "wt0": wt0, "wt1": wt1, "bb": bb})
    return in_maps


def kernel(x, noise, router_w, router_b, noise_w, noise_b, expert_w, expert_b,
           _trace=False):
    from concourse.bass_utils import run_bass_kernel_spmd

    x = np.asarray(x, dtype=np.float32)
    nc = get_program()
    in_maps = make_in_maps(x, np.asarray(noise_w), np.asarray(noise_b),
                           np.asarray(expert_w), np.asarray(expert_b))
    res = run_bass_kernel_spmd(nc, in_maps, core_ids=list(range(NCORES)),
                               trace=_trace)
    out = np.concatenate([r["out"] for r in res.results], axis=0)
    if _trace:
        kernel.last_results = res
    return out
